# revision 8
# baseline (speedup 1.0000x reference)
"""Trainium2 Bass kernel for nn_AutoregressivePrior.

Computes a K-step tiny-LSTM autoregressive prior (HID=256), projects each
step's hidden state to (loc, scale) rows of width 64, and materializes the
batch-broadcast output [K*batch_size, 64] for both loc and scale.

Strategy (8 NeuronCores, SPMD):
  - The LSTM recurrence + projections are tiny and replicated on every core.
  - The broadcast/repeat over batch_size (the memory-bound part) is sharded:
    each core writes its own batch_size/8 = 4096-row slice of every output
    row k, as one contiguous ~1 MB DMA per k.

Design notes:
  - All weights and the streaming outputs are fp16: this halves both the
    weight-load traffic (~1.2 MB/core) and the dominant output-store traffic
    (7.3 MB/core), and fp16 (11-bit mantissa, values here are O(0.1)) keeps
    end-to-end error ~1e-3 against the fp32 reference. The final upcast to
    fp32 happens on the host during unshard. Gate/projection biases stay
    fp32 (separate small tensor) and all PSUM accumulation is fp32.
  - The LSTM state lives in column layout [128 partitions, pairs]: gate
    pre-activations are computed as gates^T with the weight chunk as the
    stationary matmul operand, so every elementwise/activation op runs on
    128 lanes, and the hidden state needs no transpose between steps.
  - Every value is kept as duplicated column pairs [v0 v0 v1 v1]: a moving
    free dim >= 2 keeps the PE happy, and h then comes out of the
    elementwise chain pre-duplicated as the next step's moving operand.
  - fp16 matmuls enable FastWeightLoad on the 128x128 stationary chunks,
    roughly halving the LDWEIGHTS cost that dominates each LSTM step.
  - After step 0, x and h are both h_new, so gates = (W_ih + W_hh) @ h + b.
  - Gate chunks are ordered (g, i, f, o) and land in separate PSUM tiles:
    tanh(g)/sigmoid(i)/sigmoid(f) start as soon as their own chunk's
    matmuls finish (pipelined against the remaining gate matmuls), and the
    o chunk - only needed for the final h multiply - overlaps the c chain.
  - The projection + 128-partition broadcast are fused into one PSUM group
    by replicating the x column across the stationary operand's free dim.
  - Weight loads ride the scalar-engine HWDGE queue while output stores
    ride the sync-engine queue: the SDMA engines round-robin between the
    two rings, so row stores never queue behind the remaining weight bytes.
  - Each output row's store repeats a [128, 2 KB] SBUF block via the DMA
    read-AP; 2 KB descriptors keep the store near line rate while the
    on-chip widen stays one short DVE op.
"""

import numpy as np

import concourse.bacc as bacc
import concourse.mybir as mybir
from concourse.tile import TileContext
from concourse.bass_utils import run_bass_kernel_spmd

F32 = mybir.dt.float32
F16 = mybir.dt.float16

HID = 256
K = 7
BATCH = 32768
NCORES = 8
BS = BATCH // NCORES  # 4096 batch rows per core
P = 128               # partitions
RPP = BS // P         # 32 batch rows per partition
ZM = 64               # zm_size
REP_SB = 32           # batch-row copies materialized in SBUF per output row
REP_DMA = RPP // REP_SB  # additional repeats done by the store's read-AP

# --- packed const layouts (column offsets) ---
# megaA (f16): projection/broadcast weights + zm1 column
MA_WL = 0                  # wlst chunks (c p n): cols [0, 256)
MA_ZC = 256                # zm1 column form, duplicated pairs: [256, 260)
MA_W = 260
# biasF (f32): gate bias columns + projection bias broadcast
BF_G = 0                   # gate bias columns (g,i,f,o), duplicated: [0, 16)
BF_LS = 16                 # biasls broadcast to all partitions: [16, 144)
BF_W = 144
# megaB1 (f16): step-1 weights + step-1 input column
MB1_W0 = 0                 # w0t chunks: [0, 2048)
MB1_ZC = 2048              # zm1 column form, duplicated pairs: [2048, 2052)
MB1_W = 2052
# megaB2 (f16): steady-state weights
MB2_WS = 0                 # wst chunks: [0, 2048)
MB2_W = 2048

_NC_CACHE = {}


def build_nc():
    nc = bacc.Bacc("TRN2", target_bir_lowering=False, debug=False)

    megaA_d = nc.declare_dram_parameter("megaA", [P, MA_W], F16, isOutput=False)
    biasF_d = nc.declare_dram_parameter("biasF", [P, BF_W], F32, isOutput=False)
    megaB1_d = nc.declare_dram_parameter("megaB1", [P, MB1_W], F16, isOutput=False)
    megaB2_d = nc.declare_dram_parameter("megaB2", [P, MB2_W], F16, isOutput=False)
    out_d = nc.declare_dram_parameter("out", [K, 2, BS, ZM], F16, isOutput=True)

    with TileContext(nc) as tc:
        with (
            tc.tile_pool(name="const", bufs=1) as cpool,
            tc.tile_pool(name="state", bufs=3) as spool,
            tc.tile_pool(name="hcol", bufs=3) as hpool,
            tc.tile_pool(name="wide", bufs=8) as wpool,
            tc.tile_pool(name="prow", bufs=3) as ppool,
            tc.tile_pool(name="pgg", bufs=1, space="PSUM") as pgg_pool,
            tc.tile_pool(name="pgi", bufs=1, space="PSUM") as pgi_pool,
            tc.tile_pool(name="pgf", bufs=1, space="PSUM") as pgf_pool,
            tc.tile_pool(name="pgo", bufs=1, space="PSUM") as pgo_pool,
            tc.tile_pool(name="pbcast", bufs=3, space="PSUM") as pb_pool,
        ):
            # Row 0 needs only megaA+biasF: load them on the gpsimd (SWDGE)
            # queue while the big LSTM weights ride the scalar-engine HWDGE
            # queue - both start immediately and neither blocks the other,
            # so row-0's store and LSTM step 1 both start as early as
            # possible. Output stores use the sync + scalar HWDGE rings.
            ma = cpool.tile([P, MA_W], F16)
            nc.gpsimd.dma_start(out=ma[:], in_=megaA_d[:])
            bf = cpool.tile([P, BF_W], F32)
            nc.gpsimd.dma_start(out=bf[:], in_=biasF_d[:])
            mb1 = cpool.tile([P, MB1_W], F16)
            nc.scalar.dma_start(out=mb1[:], in_=megaB1_d[:])
            mb2 = cpool.tile([P, MB2_W], F16)
            nc.scalar.dma_start(out=mb2[:], in_=megaB2_d[:])

            wlst_sb = ma[:, MA_WL : MA_WL + 256]
            zm1c_a = ma[:, MA_ZC : MA_ZC + 4]
            bg_g, bg_i = bf[:, BF_G : BF_G + 4], bf[:, BF_G + 4 : BF_G + 8]
            bg_f, bg_o = bf[:, BF_G + 8 : BF_G + 12], bf[:, BF_G + 12 : BF_G + 16]
            blsb = bf[:, BF_LS : BF_LS + 128]

            def emit_row(k, xcr):
                """Project p_z[k] (f16 column form xcr) to loc|scale and write
                this core's batch-broadcast slice of output row k.

                The projection and 128-partition broadcast are fused into one
                PSUM accumulation group: the x column is replicated across the
                stationary operand's free dim, so every output partition
                computes the same (loc | scale) row. The projection bias is
                added by the mid-widen DVE op. High scheduler priority keeps
                later rows' matmuls from being deferred behind all remaining
                gate matmuls (which would starve the output DMA)."""
                pb = pb_pool.tile([P, 2 * ZM], F32)
                with tc.high_priority():
                    nc.tensor.matmul(
                        pb[:], lhsT=xcr[:, 0:1].broadcast_to((P, P)),
                        rhs=wlst_sb[:, 0:128], start=True, stop=False,
                    )
                    nc.tensor.matmul(
                        pb[:], lhsT=xcr[:, 2:3].broadcast_to((P, P)),
                        rhs=wlst_sb[:, 128:256], start=False, stop=True,
                    )
                # Mid-widen in three DVE ops: a tiny f32 bias-add out of PSUM
                # that also casts to fp16, then one pure fp16 broadcast copy
                # per output tensor, each running in the DVE's packed 16-bit
                # fast mode. Splitting per-t lets the loc half store as soon
                # as its copy lands, and the two halves ride the two physical
                # HWDGE rings (sync + scalar) so they drain in parallel.
                pbb = ppool.tile([P, 2 * ZM], F16)
                nc.vector.tensor_add(out=pbb[:], in0=pb[:], in1=blsb[:])
                for t, eng in ((0, nc.sync), (1, nc.scalar)):
                    midw = wpool.tile([P, REP_SB * ZM], F16)
                    nc.vector.tensor_copy(
                        out=midw[:].rearrange("p (r j) -> p r j", r=REP_SB),
                        in_=pbb[:, t * ZM : (t + 1) * ZM][
                            :, None, :
                        ].broadcast_to((P, REP_SB, ZM)),
                    )
                    eng.dma_start(
                        out=out_d[k, t].rearrange("(p s) j -> p (s j)", p=P),
                        in_=midw[:],
                    )

            def mm_chunks(dst, wsb, wofs, m0, xcr):
                """Accumulate gate chunks m0, m0+1 of W.T @ x into dst [P, 4]."""
                for dm in (0, 1):
                    m = m0 + dm
                    for c in (0, 1):
                        nc.tensor.matmul(
                            dst[:, 2 * dm : 2 * dm + 2],
                            lhsT=wsb[:, wofs + c * 1024 + m * 128 : wofs + c * 1024 + (m + 1) * 128],
                            rhs=xcr[:, 2 * c : 2 * c + 2],
                            start=(c == 0), stop=(c == 1),
                        )

            def emit_step(t, xcr_prev, st_prev):
                """One LSTM cell step, duplicated-pair column layout.

                Gate chunk order (g, i, f, o): tanh(g) starts after only 4 of
                the 16 gate matmuls, sigmoid(i) after 8, sigmoid(f) after 12 -
                the activation/elementwise chain pipelines against the gate
                matmuls instead of waiting for all of them. The o chunk is
                only needed for the final h multiply and overlaps the c chain.

                st tiles hold [tanh(g) (0:4) | c (4:8)].
                Returns (st_next, h16); h16 is [128, 4] = [h0 h0 h1 h1]."""
                wsb = mb1 if t == 1 else mb2
                wofs = MB1_W0 if t == 1 else MB2_WS
                pgG = pgg_pool.tile([P, 4], F32)
                pgI = pgi_pool.tile([P, 4], F32)
                pgF = pgf_pool.tile([P, 4], F32)
                pgO = pgo_pool.tile([P, 4], F32)
                mm_chunks(pgG, wsb, wofs, 0, xcr_prev)
                mm_chunks(pgI, wsb, wofs, 2, xcr_prev)
                mm_chunks(pgF, wsb, wofs, 4, xcr_prev)
                mm_chunks(pgO, wsb, wofs, 6, xcr_prev)

                bg = spool.tile([P, 4], F32)
                nc.vector.tensor_add(out=bg[:], in0=pgG[:], in1=bg_g)
                nc.scalar.activation(
                    out=st_prev[:, 0:4], in_=bg[:],
                    func=mybir.ActivationFunctionType.Tanh,
                )
                bi = spool.tile([P, 4], F32)
                nc.vector.tensor_add(out=bi[:], in0=pgI[:], in1=bg_i)
                si = spool.tile([P, 4], F32)
                nc.scalar.activation(
                    out=si[:], in_=bi[:],
                    func=mybir.ActivationFunctionType.Sigmoid,
                )
                st_next = spool.tile([P, 8], F32, tag="st")
                bf_ = spool.tile([P, 4], F32)
                nc.vector.tensor_add(out=bf_[:], in0=pgF[:], in1=bg_f)
                sf = spool.tile([P, 4], F32)
                nc.scalar.activation(
                    out=sf[:], in_=bf_[:],
                    func=mybir.ActivationFunctionType.Sigmoid,
                )
                if t == 1:
                    # c0 = 0: c1 = i*tanh(g) directly into st_next's c half
                    nc.vector.tensor_mul(
                        out=st_next[:, 4:8], in0=si[:], in1=st_prev[:, 0:4]
                    )
                else:
                    t1 = spool.tile([P, 4], F32)
                    nc.vector.tensor_mul(out=t1[:], in0=si[:], in1=st_prev[:, 0:4])
                    t2 = spool.tile([P, 4], F32)
                    nc.vector.tensor_mul(out=t2[:], in0=sf[:], in1=st_prev[:, 4:8])
                    nc.vector.tensor_add(out=st_next[:, 4:8], in0=t1[:], in1=t2[:])
                tc_ = spool.tile([P, 4], F32)
                nc.scalar.activation(
                    out=tc_[:], in_=st_next[:, 4:8],
                    func=mybir.ActivationFunctionType.Tanh,
                )
                # o path, concurrent with the c chain
                bo = spool.tile([P, 4], F32)
                nc.vector.tensor_add(out=bo[:], in0=pgO[:], in1=bg_o)
                so = spool.tile([P, 4], F32)
                nc.scalar.activation(
                    out=so[:], in_=bo[:],
                    func=mybir.ActivationFunctionType.Sigmoid,
                )
                h16 = hpool.tile([P, 4], F16)
                nc.vector.tensor_mul(out=h16[:], in0=so[:], in1=tc_[:])
                return st_next, h16

            emit_row(0, zm1c_a)
            xcr = mb1[:, MB1_ZC : MB1_ZC + 4]
            st = spool.tile([P, 8], F32, tag="st")
            for t in range(1, K):
                st, xcr = emit_step(t, xcr, st)
                emit_row(t, xcr)

    nc.compile()
    return nc


def _get_nc():
    if "nc" not in _NC_CACHE:
        _NC_CACHE["nc"] = build_nc()
    return _NC_CACHE["nc"]


def prepare_inputs(**inputs):
    """Host-side prep: pure numpy reshuffling of the full inputs into the
    per-core input map (identical on every core)."""
    f = lambda k: np.asarray(inputs[k], dtype=np.float32)
    zm_1, W_ih, W_hh = f("zm_1"), f("W_ih"), f("W_hh")
    b_ih, b_hh = f("b_ih"), f("b_hh")
    W_loc, b_loc, W_scale, b_scale = f("W_loc"), f("b_loc"), f("W_scale"), f("b_scale")
    assert int(inputs["K"]) == K and int(inputs["batch_size"]) == BATCH

    def cpn(wt):
        # [256, N] -> chunked [128, 2*N]: chunk c (rows c*128..) at cols [c*N, (c+1)*N)
        n = wt.shape[1]
        return wt.reshape(2, P, n).transpose(1, 0, 2).reshape(P, 2 * n)

    # reorder gates (i, f, g, o) -> (g, i, f, o): g starts the serial chain,
    # i and f pipeline behind it, o overlaps the c chain
    perm = np.r_[512:768, 0:256, 256:512, 768:1024]
    w0t = W_ih[perm].T                 # [256, 1024]
    wst = (W_ih + W_hh)[perm].T        # [256, 1024]
    biasg = (b_ih + b_hh)[perm]        # [1024]
    wlst = np.concatenate([W_loc.T, W_scale.T], axis=1)  # [256, 128]
    biasls = np.concatenate([b_loc, b_scale])            # [128]
    zm1c = zm_1.reshape(2, P).T                          # [128, 2]
    zm1c_dup = np.repeat(zm1c, 2, axis=1)                # [128, 4]

    ma = np.zeros((P, MA_W), np.float16)
    ma[:, MA_WL : MA_WL + 256] = cpn(wlst).astype(np.float16)
    ma[:, MA_ZC : MA_ZC + 4] = zm1c_dup.astype(np.float16)

    bfr = np.zeros((P, BF_W), np.float32)
    bfr[:, BF_G : BF_G + 16] = np.repeat(biasg.reshape(8, P).T, 2, axis=1)
    bfr[:, BF_LS : BF_LS + 128] = biasls[None, :]

    mb1 = np.zeros((P, MB1_W), np.float16)
    mb1[:, MB1_W0 : MB1_W0 + 2048] = cpn(w0t).astype(np.float16)
    mb1[:, MB1_ZC : MB1_ZC + 4] = zm1c_dup.astype(np.float16)

    mb2 = np.zeros((P, MB2_W), np.float16)
    mb2[:, MB2_WS : MB2_WS + 2048] = cpn(wst).astype(np.float16)

    return {"megaA": ma, "biasF": bfr, "megaB1": mb1, "megaB2": mb2}


def execute(in_map, **kwargs):
    nc = _get_nc()
    return run_bass_kernel_spmd(
        nc, [dict(in_map) for _ in range(NCORES)], core_ids=list(range(NCORES)), **kwargs
    )


def assemble_output(results):
    loc = np.empty((K, BATCH, ZM), np.float32)
    scale = np.empty((K, BATCH, ZM), np.float32)
    for c in range(NCORES):
        o = results[c]["out"]  # [K, 2, BS, ZM] fp16
        loc[:, c * BS : (c + 1) * BS] = o[:, 0]
        scale[:, c * BS : (c + 1) * BS] = o[:, 1]
    return loc.reshape(-1, ZM), scale.reshape(-1, ZM)


def kernel(**inputs):
    in_map = prepare_inputs(**inputs)
    res = execute(in_map)
    return assemble_output(res.results)


# revision 9
# speedup vs baseline: 1.3311x; 1.3311x over previous
"""Trainium2 Bass kernel for nn_AutoregressivePrior.

Computes a K-step tiny-LSTM autoregressive prior (HID=256), projects each
step's hidden state to (loc, scale) rows of width 64, and materializes the
batch-broadcast output [K*batch_size, 64] for both loc and scale.

Strategy (8 NeuronCores, SPMD):
  - The LSTM recurrence + projections are tiny and replicated on every core.
  - The broadcast/repeat over batch_size (the memory-bound part) is sharded:
    each core writes its own batch_size/8 = 4096-row slice of every output
    row k, as one contiguous ~1 MB DMA per k.

Design notes:
  - All weights and the streaming outputs are fp16: this halves both the
    weight-load traffic (~1.2 MB/core) and the dominant output-store traffic
    (7.3 MB/core), and fp16 (11-bit mantissa, values here are O(0.1)) keeps
    end-to-end error ~4e-4 against the fp32 reference. The final upcast to
    fp32 happens on the host during unshard. Gate/projection biases stay
    fp32 (separate small tensor) and all PSUM accumulation is fp32.
  - The LSTM state lives in column layout [128 partitions, pairs]: gate
    pre-activations are computed as gates^T with the weight chunk as the
    stationary matmul operand, so every elementwise/activation op runs on
    128 lanes, and the hidden state needs no transpose between steps.
  - Every value is kept as duplicated column pairs [v0 v0 v1 v1]: a moving
    free dim >= 2 keeps the PE happy, and h then comes out of the
    elementwise chain pre-duplicated as the next step's moving operand.
  - fp16 matmuls enable FastWeightLoad on the 128x128 stationary chunks,
    roughly halving the LDWEIGHTS cost that dominates each LSTM step.
  - After step 0, x and h are both h_new, so gates = (W_ih + W_hh) @ h + b.
  - Gates are paired (g,i) and (f,o) into two PSUM tiles: one DVE bias-add
    and at most two activations per pair, so the whole post-matmul chain is
    ~10 engine ops per step. tanh(g)/sigmoid(i) start after only half the
    gate matmuls; sigmoid(f,o) lands right when the c-chain needs it.
  - Step 1's weights are packed (g,i) block first, (f,o) block second, and
    loaded as two DMAs, so the recurrence starts as soon as the first half
    of W_ih is resident. All weight loads ride the scalar-engine HWDGE
    queue; output stores ride the sync-engine queue, so stores never wait
    behind weight bytes. The scalar engine only issues its DMAs before its
    first activation, keeping the ACT chain (on the recurrence critical
    path) clean.
  - The projection + 128-partition broadcast are fused into one PSUM group
    by replicating the x column across the stationary operand's free dim.
  - Widening is two DVE ops: a tiny f32 bias-add out of PSUM that casts to
    fp16, then one fp16->fp16 broadcast copy in the packed 16-bit fast
    mode, materializing the full [128, 4 KB] store block in SBUF. Stores
    are plain contiguous 2-D APs with 4 KB descriptors (~line rate).
  - Row 0's widen+store is split into loc/scale halves so its first bytes
    hit the queue ~0.7 us earlier (that store opens the DMA pipeline).
"""

import numpy as np

import concourse.bacc as bacc
import concourse.mybir as mybir
from concourse.tile import TileContext
from concourse.bass_utils import run_bass_kernel_spmd

F32 = mybir.dt.float32
F16 = mybir.dt.float16

HID = 256
K = 7
BATCH = 32768
NCORES = 8
BS = BATCH // NCORES  # 4096 batch rows per core
P = 128               # partitions
RPP = BS // P         # 32 batch rows per partition
ZM = 64               # zm_size

# --- packed const layouts (column offsets) ---
# megaA (f16): projection/broadcast weights + zm1 column
MA_WL = 0                  # wlst chunks (c p n): cols [0, 256)
MA_ZC = 256                # zm1 column form, duplicated pairs: [256, 260)
MA_W = 260
# biasF (f32): gate bias columns + projection bias broadcast
BF_G = 0                   # gate bias columns (g,i,f,o), duplicated: [0, 16)
BF_LS = 16                 # biasls broadcast to all partitions: [16, 144)
BF_W = 144
# megaB1 (f16): step-1 weights, (g,i) block then (f,o) block
MB1_W = 2048
# megaB2 (f16): steady-state weights
MB2_W = 2048

_NC_CACHE = {}


def _mb1_col(m, c):
    # step-1 layout: m-chunks 0-3 (g,i) in cols [0,1024), 4-7 (f,o) above
    return (0 if m < 4 else 1024) + (m % 4) * 128 + c * 512


def _mb2_col(m, c):
    return c * 1024 + m * 128


def build_nc():
    nc = bacc.Bacc("TRN2", target_bir_lowering=False, debug=False)

    megaA_d = nc.declare_dram_parameter("megaA", [P, MA_W], F16, isOutput=False)
    biasF_d = nc.declare_dram_parameter("biasF", [P, BF_W], F32, isOutput=False)
    megaB1_d = nc.declare_dram_parameter("megaB1", [P, MB1_W], F16, isOutput=False)
    megaB2_d = nc.declare_dram_parameter("megaB2", [P, MB2_W], F16, isOutput=False)
    out_d = nc.declare_dram_parameter("out", [K, 2, BS, ZM], F16, isOutput=True)

    with TileContext(nc) as tc:
        with (
            tc.tile_pool(name="const", bufs=1) as cpool,
            tc.tile_pool(name="state", bufs=3) as spool,
            tc.tile_pool(name="hcol", bufs=3) as hpool,
            tc.tile_pool(name="wide", bufs=5) as wpool,
            tc.tile_pool(name="prow", bufs=3) as ppool,
            tc.tile_pool(name="pgi", bufs=1, space="PSUM") as pgi_pool,
            tc.tile_pool(name="pfo", bufs=1, space="PSUM") as pfo_pool,
            tc.tile_pool(name="pbcast", bufs=3, space="PSUM") as pb_pool,
        ):
            # Load order on the scalar queue: megaA (row-0 projection + zm1
            # column), step-1 (g,i) weights, biases, step-1 (f,o) weights,
            # steady-state weights. Row 0's store and LSTM step 1 both start
            # as early as the queue can feed them.
            ma = cpool.tile([P, MA_W], F16)
            nc.scalar.dma_start(out=ma[:], in_=megaA_d[:])
            mb1 = cpool.tile([P, MB1_W], F16)
            nc.scalar.dma_start(out=mb1[:, 0:1024], in_=megaB1_d[:, 0:1024])
            bf = cpool.tile([P, BF_W], F32)
            nc.scalar.dma_start(out=bf[:], in_=biasF_d[:])
            nc.scalar.dma_start(out=mb1[:, 1024:2048], in_=megaB1_d[:, 1024:2048])
            mb2 = cpool.tile([P, MB2_W], F16)
            nc.scalar.dma_start(out=mb2[:], in_=megaB2_d[:])

            wlst_sb = ma[:, MA_WL : MA_WL + 256]
            zm1c_a = ma[:, MA_ZC : MA_ZC + 4]
            bg_gi = bf[:, BF_G : BF_G + 8]
            bg_fo = bf[:, BF_G + 8 : BF_G + 16]
            blsb = bf[:, BF_LS : BF_LS + 128]

            def emit_row(k, xcr, split=False):
                """Project p_z[k] (f16 column form xcr) to loc|scale and write
                this core's batch-broadcast slice of output row k.

                The projection and 128-partition broadcast are fused into one
                PSUM accumulation group: the x column is replicated across the
                stationary operand's free dim, so every output partition
                computes the same (loc | scale) row. High scheduler priority
                keeps later rows' matmuls from being deferred behind gate
                matmuls (which would starve the output DMA)."""
                pb = pb_pool.tile([P, 2 * ZM], F32)
                with tc.high_priority():
                    nc.tensor.matmul(
                        pb[:], lhsT=xcr[:, 0:1].broadcast_to((P, P)),
                        rhs=wlst_sb[:, 0:128], start=True, stop=False,
                    )
                    nc.tensor.matmul(
                        pb[:], lhsT=xcr[:, 2:3].broadcast_to((P, P)),
                        rhs=wlst_sb[:, 128:256], start=False, stop=True,
                    )
                pbb = ppool.tile([P, 2 * ZM], F16)
                nc.vector.tensor_add(out=pbb[:], in0=pb[:], in1=blsb[:])
                if split:
                    for t in (0, 1):
                        midw = wpool.tile([P, RPP * ZM], F16)
                        nc.vector.tensor_copy(
                            out=midw[:].rearrange("p (r j) -> p r j", r=RPP),
                            in_=pbb[:, t * ZM : (t + 1) * ZM][
                                :, None, :
                            ].broadcast_to((P, RPP, ZM)),
                        )
                        nc.sync.dma_start(
                            out=out_d[k, t].rearrange("(p s) j -> p (s j)", p=P),
                            in_=midw[:],
                        )
                else:
                    midw = wpool.tile([P, 2 * RPP * ZM], F16)
                    nc.vector.tensor_copy(
                        out=midw[:].rearrange("p (t r j) -> p t r j", t=2, r=RPP),
                        in_=pbb[:].rearrange("p (t j) -> p t j", t=2)[
                            :, :, None, :
                        ].broadcast_to((P, 2, RPP, ZM)),
                    )
                    nc.sync.dma_start(
                        out=out_d[k].rearrange("t (p s) j -> p t (s j)", p=P, s=RPP),
                        in_=midw[:].rearrange("p (t sj) -> p t sj", t=2),
                    )

            def emit_step(t, xcr_prev, st_prev):
                """One LSTM cell step, duplicated-pair column layout.

                Gates land pairwise in two PSUM tiles: (g,i) then (f,o).
                tanh(g)/sigmoid(i) start after only half the gate matmuls;
                sigmoid over the whole (f,o) tile yields f for the c-chain
                and o (needed only for the final h multiply) in one op.

                st tiles hold [tanh(g) (0:4) | c (4:8)].
                Returns (st_next, h16); h16 is [128, 4] = [h0 h0 h1 h1]."""
                wsb = mb1 if t == 1 else mb2
                colf = _mb1_col if t == 1 else _mb2_col
                pGI = pgi_pool.tile([P, 8], F32)
                pFO = pfo_pool.tile([P, 8], F32)
                for dst, mbase in ((pGI, 0), (pFO, 4)):
                    for dm in range(4):
                        m = mbase + dm
                        for c in (0, 1):
                            col = colf(m, c)
                            nc.tensor.matmul(
                                dst[:, 2 * dm : 2 * dm + 2],
                                lhsT=wsb[:, col : col + 128],
                                rhs=xcr_prev[:, 2 * c : 2 * c + 2],
                                start=(c == 0), stop=(c == 1),
                            )

                gi = spool.tile([P, 8], F32)
                nc.vector.tensor_add(out=gi[:], in0=pGI[:], in1=bg_gi)
                nc.scalar.activation(
                    out=st_prev[:, 0:4], in_=gi[:, 0:4],
                    func=mybir.ActivationFunctionType.Tanh,
                )
                si = spool.tile([P, 4], F32)
                nc.scalar.activation(
                    out=si[:], in_=gi[:, 4:8],
                    func=mybir.ActivationFunctionType.Sigmoid,
                )
                fo = spool.tile([P, 8], F32)
                nc.vector.tensor_add(out=fo[:], in0=pFO[:], in1=bg_fo)
                sfo = spool.tile([P, 8], F32)
                nc.scalar.activation(
                    out=sfo[:], in_=fo[:],
                    func=mybir.ActivationFunctionType.Sigmoid,
                )
                st_next = spool.tile([P, 8], F32, tag="st")
                if t == 1:
                    # c0 = 0: c1 = i*tanh(g) directly into st_next's c half
                    nc.vector.tensor_mul(
                        out=st_next[:, 4:8], in0=si[:], in1=st_prev[:, 0:4]
                    )
                else:
                    t1 = spool.tile([P, 4], F32)
                    nc.vector.tensor_mul(out=t1[:], in0=si[:], in1=st_prev[:, 0:4])
                    t2 = spool.tile([P, 4], F32)
                    nc.vector.tensor_mul(
                        out=t2[:], in0=sfo[:, 0:4], in1=st_prev[:, 4:8]
                    )
                    nc.vector.tensor_add(out=st_next[:, 4:8], in0=t1[:], in1=t2[:])
                tc_ = spool.tile([P, 4], F32)
                nc.scalar.activation(
                    out=tc_[:], in_=st_next[:, 4:8],
                    func=mybir.ActivationFunctionType.Tanh,
                )
                h16 = hpool.tile([P, 4], F16)
                nc.vector.tensor_mul(out=h16[:], in0=sfo[:, 4:8], in1=tc_[:])
                return st_next, h16

            emit_row(0, zm1c_a, split=True)
            xcr = zm1c_a
            st = spool.tile([P, 8], F32, tag="st")
            for t in range(1, K):
                st, xcr = emit_step(t, xcr, st)
                emit_row(t, xcr)

    nc.compile()
    return nc


def _get_nc():
    if "nc" not in _NC_CACHE:
        _NC_CACHE["nc"] = build_nc()
    return _NC_CACHE["nc"]


def prepare_inputs(**inputs):
    """Host-side prep: pure numpy reshuffling of the full inputs into the
    per-core input map (identical on every core)."""
    f = lambda k: np.asarray(inputs[k], dtype=np.float32)
    zm_1, W_ih, W_hh = f("zm_1"), f("W_ih"), f("W_hh")
    b_ih, b_hh = f("b_ih"), f("b_hh")
    W_loc, b_loc, W_scale, b_scale = f("W_loc"), f("b_loc"), f("W_scale"), f("b_scale")
    assert int(inputs["K"]) == K and int(inputs["batch_size"]) == BATCH

    # reorder gates (i, f, g, o) -> (g, i, f, o): g starts the serial chain,
    # i pipelines behind it, f/o feed the c-chain and final h multiply
    perm = np.r_[512:768, 0:256, 256:512, 768:1024]
    w0t = W_ih[perm].T                 # [256, 1024]
    wst = (W_ih + W_hh)[perm].T        # [256, 1024]
    biasg = (b_ih + b_hh)[perm]        # [1024]
    wlst = np.concatenate([W_loc.T, W_scale.T], axis=1)  # [256, 128]
    biasls = np.concatenate([b_loc, b_scale])            # [128]
    zm1c = zm_1.reshape(2, P).T                          # [128, 2]
    zm1c_dup = np.repeat(zm1c, 2, axis=1)                # [128, 4]

    def chunk(wt, m, c):
        # stationary lhsT for gate m-chunk m, x-chunk c: [128, 128]
        return wt[c * 128 : (c + 1) * 128, m * 128 : (m + 1) * 128]

    def pack(wt, colf):
        out = np.zeros((P, 2048), np.float16)
        for m in range(8):
            for c in range(2):
                col = colf(m, c)
                out[:, col : col + 128] = chunk(wt, m, c).astype(np.float16)
        return out

    def cpn(wt):
        # [256, N] -> chunked [128, 2*N]: chunk c (rows c*128..) at cols [c*N, (c+1)*N)
        n = wt.shape[1]
        return wt.reshape(2, P, n).transpose(1, 0, 2).reshape(P, 2 * n)

    ma = np.zeros((P, MA_W), np.float16)
    ma[:, MA_WL : MA_WL + 256] = cpn(wlst).astype(np.float16)
    ma[:, MA_ZC : MA_ZC + 4] = zm1c_dup.astype(np.float16)

    bfr = np.zeros((P, BF_W), np.float32)
    bfr[:, BF_G : BF_G + 16] = np.repeat(biasg.reshape(8, P).T, 2, axis=1)
    bfr[:, BF_LS : BF_LS + 128] = biasls[None, :]

    mb1 = pack(w0t, _mb1_col)
    mb2 = pack(wst, _mb2_col)

    return {"megaA": ma, "biasF": bfr, "megaB1": mb1, "megaB2": mb2}


def execute(in_map, **kwargs):
    nc = _get_nc()
    return run_bass_kernel_spmd(
        nc, [dict(in_map) for _ in range(NCORES)], core_ids=list(range(NCORES)), **kwargs
    )


def assemble_output(results):
    loc = np.empty((K, BATCH, ZM), np.float32)
    scale = np.empty((K, BATCH, ZM), np.float32)
    for c in range(NCORES):
        o = results[c]["out"]  # [K, 2, BS, ZM] fp16
        loc[:, c * BS : (c + 1) * BS] = o[:, 0]
        scale[:, c * BS : (c + 1) * BS] = o[:, 1]
    return loc.reshape(-1, ZM), scale.reshape(-1, ZM)


def kernel(**inputs):
    in_map = prepare_inputs(**inputs)
    res = execute(in_map)
    return assemble_output(res.results)
